# revision 1
# baseline (speedup 1.0000x reference)
"""TRN2 Bass kernel: CTC compressor (greedy-path run-length segmentation +
per-segment weighted mean), data-parallel over 8 NeuronCores.

kernel(hidden_states (16,2048,512) f32, ctc_logits (16,2048,4096) f32,
       lengths (16,) i32) -> (compressed (16,2048,512) f32,
                              new_lengths (16,) i32)

Per-core program (2 batch rows per core, no collectives):
  1. Stream logits in (128 frames x V) tiles: ACT computes exp(l) with an
     accumulated row sum (softmax denominator); DVE computes the max logit
     (tensor_reduce) and the exact argmax index via the accumulated sum of
     (l == max) * iota_v (single nonzero term).  The greedy token's
     probability is exp(max)/denom — no gather needed.
  2. Segment boundaries: start[t] = (pred[t] != pred[t-1]) & (t < len);
     the partition-direction shift runs as a PE subdiagonal matmul; segment
     ids come from a PE triangular-ones cumsum matmul plus a K=1 broadcast
     matmul adding per-column prefix offsets.
  3. Segment sums: out[s,:] = sum_t [seg(t)==s] * p_t * [1|h_t] as PE
     matmuls against on-the-fly 0/1 indicator tiles (seg[t] <= t skips the
     upper triangle).  X = p*[1|h] is split into bf16 hi + bf16 residual for
     fp32-grade accuracy at 1 PE cycle/row; the ones column accumulates
     segp, used to normalize the weighted mean from PSUM on ACT.
"""

import numpy as np

B, T, D, V = 16, 2048, 512, 4096
N_CORES = 8
B_PC = B // N_CORES

_CACHE = {}


def _build():
    import concourse.bacc as bacc
    import concourse.tile as tile
    from concourse import mybir

    F32 = mybir.dt.float32
    BF16 = mybir.dt.bfloat16
    I32 = mybir.dt.int32
    U16 = mybir.dt.uint16
    A = mybir.AluOpType
    AF = mybir.ActivationFunctionType

    B_pc, KT, P = B_PC, T // 128, 128
    DW = D + 1
    DA = 257  # [ones|h] split: cols 0:257 -> psum bank 0, rest -> bank 1

    nc = bacc.Bacc("TRN2", target_bir_lowering=False, debug=False,
                   num_devices=N_CORES)

    hs = nc.declare_dram_parameter("hs", [B_pc, T, D], F32, isOutput=False)
    lg = nc.declare_dram_parameter("lg", [B_pc, T, V], F32, isOutput=False)
    ln = nc.declare_dram_parameter("ln", [B_pc, 1], I32, isOutput=False)
    out = nc.declare_dram_parameter("out", [B_pc, T, D], F32, isOutput=True)
    nl = nc.declare_dram_parameter("nl", [B_pc, 1], I32, isOutput=True)

    with tile.TileContext(nc) as tc:
        with (
            tc.tile_pool(name="singles", bufs=1) as singles,
            tc.tile_pool(name="lg_pool", bufs=3) as lg_pool,
            tc.tile_pool(name="hid_pool", bufs=2) as hid_pool,
            tc.tile_pool(name="xhi_pool", bufs=2) as xhi_pool,
            tc.tile_pool(name="xlo_pool", bufs=2) as xlo_pool,
            tc.tile_pool(name="cols", bufs=2) as cols_pool,
            tc.tile_pool(name="ep_pool", bufs=4) as ep_pool,
            tc.tile_pool(name="osb_pool", bufs=3) as osb_pool,
            tc.tile_pool(name="small", bufs=4) as small_pool,
            tc.tile_pool(name="psum_big", bufs=2, space="PSUM") as psum_big,
            tc.tile_pool(name="psum_small", bufs=1, space="PSUM") as psum_sm,
        ):
            # ---- constants ----
            stage_i = hid_pool.tile([P, V], I32, tag="hid_full")
            nc.gpsimd.iota(stage_i, pattern=[[1, V]], base=0,
                           channel_multiplier=0)
            iota_v_u16 = singles.tile([P, V], U16)
            nc.gpsimd.tensor_copy(out=iota_v_u16, in_=stage_i)

            stage_i2 = hid_pool.tile([P, V], I32, tag="hid_full")
            nc.gpsimd.iota(stage_i2[:, 0:KT], pattern=[[128, KT]], base=0,
                           channel_multiplier=1)
            iota_t_f = singles.tile([P, KT], F32)
            nc.gpsimd.tensor_copy(out=iota_t_f, in_=stage_i2[:, 0:KT])

            ones_pp = singles.tile([P, P], F32)
            nc.vector.memset(ones_pp, 1.0)
            U = singles.tile([P, P], F32)  # U[j,i] = 1 if i >= j
            nc.gpsimd.affine_select(
                out=U, in_=ones_pp, pattern=[[1, P]],
                compare_op=A.is_ge, fill=0.0, base=0, channel_multiplier=-1)
            ones_1p = singles.tile([1, P], F32)
            nc.vector.memset(ones_1p, 1.0)
            ones_p1 = singles.tile([P, 1], F32)
            nc.vector.memset(ones_p1, 1.0)
            zeros_1k = singles.tile([1, KT], F32)
            nc.vector.memset(zeros_1k, 0.0)
            Sshift = singles.tile([P, P], F32)  # S[j,i] = 1 iff j == i-1
            nc.gpsimd.affine_select(
                out=Sshift, in_=ones_pp, pattern=[[1, P]],
                compare_op=A.is_equal, fill=0.0, base=-1,
                channel_multiplier=-1)
            Ccorner = singles.tile([P, P], F32)  # 1 only at (j=127, i=0)
            nc.gpsimd.affine_select(
                out=Ccorner, in_=ones_pp, pattern=[[1, P]],
                compare_op=A.is_equal, fill=0.0, base=P - 1,
                channel_multiplier=-1)

            for b in range(B_pc):
                denom_cols = cols_pool.tile([P, KT], F32, tag="denom")
                gm_cols = cols_pool.tile([P, KT], F32, tag="gmax")
                idxf = cols_pool.tile([P, KT], F32, tag="idxf")
                hid_full = hid_pool.tile([P, KT, DW], F32)
                # bf16 hi/lo X tiles; their views double as dump space for
                # the full-size exp / argmax elementwise outputs.
                xhi = xhi_pool.tile([P, KT * DW], BF16)
                xlo = xlo_pool.tile([P, KT * DW], BF16)
                dump_idx = xhi[:, 0:V]
                dump_exp = xlo[:, :].bitcast(F32)[:, 0:V]

                nc.vector.memset(hid_full[:, :, 0:1], 1.0)

                # ---- phase 1: stream logits ----
                for k in range(KT):
                    r0 = k * 128
                    lg_t = lg_pool.tile([P, V], F32)
                    nc.default_dma_engine.dma_start(
                        out=lg_t, in_=lg[b, r0:r0 + 128, :])
                    nc.vector.tensor_reduce(
                        out=gm_cols[:, k:k + 1], in_=lg_t,
                        axis=mybir.AxisListType.X, op=A.max)
                    nc.vector.scalar_tensor_tensor(
                        out=dump_idx, in0=lg_t,
                        scalar=gm_cols[:, k:k + 1], in1=iota_v_u16,
                        op0=A.is_equal, op1=A.mult,
                        accum_out=idxf[:, k:k + 1])
                    nc.scalar.activation(
                        out=dump_exp, in_=lg_t, func=AF.Exp,
                        accum_out=denom_cols[:, k:k + 1])
                    nc.default_dma_engine.dma_start(
                        out=hid_full[:, k, 1:DW],
                        in_=hs[b, r0:r0 + 128, :])

                # ---- phase 2: segmentation (small ops) ----
                len_i = small_pool.tile([P, 1], I32, tag="leni")
                nc.default_dma_engine.dma_start(
                    out=len_i, in_=ln[b].to_broadcast([P, 1]))
                len_f = small_pool.tile([P, 1], F32, tag="lenf")
                nc.vector.tensor_copy(out=len_f, in_=len_i)

                valid = cols_pool.tile([P, KT], F32, tag="valid")
                nc.vector.tensor_scalar(
                    out=valid, in0=iota_t_f, scalar1=len_f, scalar2=None,
                    op0=A.is_lt)

                p_cols = cols_pool.tile([P, KT], F32, tag="pcols")
                rcp_den = cols_pool.tile([P, KT], F32, tag="rcpden")
                nc.vector.reciprocal(out=rcp_den, in_=denom_cols)
                emax = cols_pool.tile([P, KT], F32, tag="emax")
                nc.scalar.activation(out=emax, in_=gm_cols, func=AF.Exp)
                nc.vector.tensor_mul(out=p_cols, in0=emax, in1=rcp_den)
                nc.vector.tensor_mul(out=p_cols, in0=p_cols, in1=valid)

                ps_prev = psum_sm.tile([P, KT], F32, tag="ps_prev")
                nc.tensor.matmul(ps_prev, lhsT=Sshift, rhs=idxf,
                                 start=True, stop=False)
                idxf_sh = cols_pool.tile([P, KT], F32, tag="idxfsh")
                nc.vector.memset(idxf_sh[:, 0:1], 0.0)
                nc.vector.tensor_copy(
                    out=idxf_sh[:, 1:KT], in_=idxf[:, 0:KT - 1])
                nc.tensor.matmul(ps_prev, lhsT=Ccorner, rhs=idxf_sh,
                                 start=False, stop=True)
                neq = cols_pool.tile([P, KT], F32, tag="neq")
                nc.vector.tensor_tensor(
                    out=neq, in0=idxf, in1=ps_prev, op=A.not_equal)
                nc.vector.memset(neq[0:1, 0:1], 1.0)

                start_c = cols_pool.tile([P, KT], F32, tag="startc")
                nc.vector.tensor_mul(out=start_c, in0=neq, in1=valid)

                ps_cs = psum_sm.tile([1, KT], F32, tag="ps_cs")
                nc.tensor.matmul(ps_cs, lhsT=ones_p1, rhs=start_c,
                                 start=True, stop=True)
                colsum = small_pool.tile([1, KT], F32, tag="colsum")
                nc.scalar.copy(out=colsum, in_=ps_cs)
                offs_i = small_pool.tile([1, KT], F32, tag="offsi")
                nc.vector.tensor_tensor_scan(
                    out=offs_i, data0=colsum, data1=zeros_1k, initial=0.0,
                    op0=A.add, op1=A.add)
                offs_x = small_pool.tile([1, KT], F32, tag="offsx")
                nc.vector.tensor_sub(out=offs_x, in0=offs_i, in1=colsum)

                nl_i = small_pool.tile([1, 1], I32, tag="nli")
                nc.vector.tensor_copy(out=nl_i, in_=offs_i[:, KT - 1:KT])
                nc.default_dma_engine.dma_start(out=nl[b], in_=nl_i)

                ps_seg = psum_sm.tile([P, KT], F32, tag="ps_seg")
                nc.tensor.matmul(ps_seg, lhsT=U, rhs=start_c,
                                 start=True, stop=False)
                nc.tensor.matmul(ps_seg, lhsT=ones_1p, rhs=offs_x,
                                 start=False, stop=True)
                seg0 = cols_pool.tile([P, KT], F32, tag="seg0")
                nc.scalar.copy(out=seg0, in_=ps_seg)

                # ---- phase 3+4: segment sums via indicator matmuls ----
                for k in range(KT):
                    nc.scalar.activation(
                        out=xhi[:, k * DW:(k + 1) * DW],
                        in_=hid_full[:, k, :], func=AF.Copy,
                        scale=p_cols[:, k:k + 1])
                    nc.vector.scalar_tensor_tensor(
                        out=xlo[:, k * DW:(k + 1) * DW],
                        in0=hid_full[:, k, :], scalar=p_cols[:, k:k + 1],
                        in1=xhi[:, k * DW:(k + 1) * DW],
                        op0=A.mult, op1=A.subtract)
                for m in range(KT):
                    c0 = m * 128
                    psum_t = psum_big.tile([P, 1024], F32)
                    for k in range(m, KT):
                        ep = ep_pool.tile([P, P], BF16)
                        nc.vector.tensor_scalar(
                            out=ep, in0=iota_v_u16[:, 1 + c0:1 + c0 + 128],
                            scalar1=seg0[:, k:k + 1], scalar2=None,
                            op0=A.is_equal)
                        for ri, rr in enumerate((xhi, xlo)):
                            first = (k == m) and ri == 0
                            last = (k == KT - 1) and ri == 1
                            nc.tensor.matmul(
                                psum_t[:, 0:DA], lhsT=ep,
                                rhs=rr[:, k * DW:k * DW + DA],
                                start=first, stop=last)
                            nc.tensor.matmul(
                                psum_t[:, 512:512 + DW - DA], lhsT=ep,
                                rhs=rr[:, k * DW + DA:(k + 1) * DW],
                                start=first, stop=last)
                    segp = small_pool.tile([P, 1], F32, tag="segp")
                    nc.vector.tensor_scalar_add(
                        out=segp, in0=psum_t[:, 0:1], scalar1=1e-10)
                    rcp = small_pool.tile([P, 1], F32, tag="rcp")
                    nc.vector.reciprocal(out=rcp, in_=segp)
                    osb = osb_pool.tile([P, D], F32)
                    nc.scalar.activation(
                        out=osb[:, 0:DA - 1], in_=psum_t[:, 1:DA],
                        func=AF.Copy, scale=rcp)
                    nc.scalar.activation(
                        out=osb[:, DA - 1:D],
                        in_=psum_t[:, 512:512 + DW - DA],
                        func=AF.Copy, scale=rcp)
                    nc.default_dma_engine.dma_start(
                        out=out[b, c0:c0 + 128, :], in_=osb)

    nc.compile()
    return nc


def kernel(hidden_states, ctc_logits, lengths):
    from concourse.bass_utils import run_bass_kernel_spmd

    hidden_states = np.ascontiguousarray(np.asarray(hidden_states, dtype=np.float32))
    ctc_logits = np.ascontiguousarray(np.asarray(ctc_logits, dtype=np.float32))
    lengths = np.ascontiguousarray(np.asarray(lengths, dtype=np.int32))
    assert hidden_states.shape == (B, T, D)
    assert ctc_logits.shape == (B, T, V)

    if "nc" not in _CACHE:
        _CACHE["nc"] = _build()
    nc = _CACHE["nc"]

    in_maps = []
    for c in range(N_CORES):
        s = slice(c * B_PC, (c + 1) * B_PC)
        in_maps.append({
            "hs": hidden_states[s],
            "lg": ctc_logits[s],
            "ln": lengths[s].reshape(B_PC, 1),
        })
    res = run_bass_kernel_spmd(nc, in_maps, list(range(N_CORES)))
    compressed = np.concatenate(
        [res.results[c]["out"] for c in range(N_CORES)], axis=0)
    new_lengths = np.concatenate(
        [res.results[c]["nl"].reshape(-1) for c in range(N_CORES)], axis=0
    ).astype(np.int32)
    return compressed, new_lengths
